# revision 14
# baseline (speedup 1.0000x reference)
"""RBF-kernel causal attention on 8 Trainium2 NeuronCores.

B=2, H=16, N=2048, D=64. Shards the 32 (b,h) attention instances across 8
cores (4 heads per core). Math notes:

  logits = -relu(||q-k||^2)/sqrt(D); relu is a no-op (||q-k||^2 >= 0 up to
  rounding), and softmax is invariant to per-query offsets, so
      softmax_n(-(qsq_m + ksq_n - 2 qk)/8) == softmax_n(qk/4 - ksq_n/8)
  We compute P'' = exp(0.25 * K Q^T) in a [key, query] layout and fold the
  exp(-0.125 ksq_n) per-key factor into V (and into the appended ones-column
  that produces the softmax denominator):
      [O^T | l] accumulates via matmul(lhsT=V_aug_scaled, rhs=P'').
  Final output O[m,d] = OT[d,m] / l[m], un-transposed via PE transpose.

Emission is manually software-pipelined: head h+1's setup chunks (transposes,
ksq, V scaling) are interleaved between head h's query blocks so the tile
scheduler (limited lookahead) can overlap them.
"""

import sys

if "/opt/trn_rl_repo" not in sys.path:
    sys.path.insert(0, "/opt/trn_rl_repo")

import numpy as np

import concourse.bacc as bacc
import concourse.mybir as mybir
import concourse.tile as tile
from concourse.masks import make_identity

B, H, N, D = 2, 16, 2048, 64
NCORES = 8
HPC = (B * H) // NCORES  # heads per core = 4
P = 128                  # partitions
NT = N // P              # key tiles per head = 16
QB = 512                 # query block (matmul moving dim)
MBS = N // QB            # query blocks per head = 4
G = 2                    # key tiles per exp/ACT group (2 PSUM banks)

F32 = mybir.dt.float32
# float32r = relaxed-precision fp32 matmul (1 cycle/row at moving dim >= 256
# instead of 4 for float32)
MM_DT = mybir.dt.float32r


def build_nc():
    nc = bacc.Bacc("TRN2", target_bir_lowering=False, debug=False)
    q = nc.dram_tensor("q", [HPC, N, D], F32, kind="ExternalInput")
    k = nc.dram_tensor("k", [HPC, N, D], F32, kind="ExternalInput")
    v = nc.dram_tensor("v", [HPC, N, D], F32, kind="ExternalInput")
    out = nc.dram_tensor("out", [HPC, N, D], F32, kind="ExternalOutput")

    with tile.TileContext(nc) as tc:
        with (
            tc.tile_pool(name="const", bufs=1) as const_pool,
            tc.tile_pool(name="loads", bufs=1) as load_pool,
            tc.tile_pool(name="head", bufs=2) as head_pool,
            tc.tile_pool(name="work", bufs=3) as work_pool,
            tc.tile_pool(name="p", bufs=5) as p_pool,
            tc.tile_pool(name="epi", bufs=3) as epi_pool,
            tc.tile_pool(name="st", bufs=3, space="PSUM") as st_pool,
            tc.tile_pool(name="otp", bufs=2, space="PSUM") as ot_pool,
        ):
            identity = const_pool.tile([P, P], F32)
            make_identity(nc, identity)

            # prefetch every head's inputs up front: no-wait DMAs stream in
            # the background while compute proceeds
            knats, qnats, vtmps = [], [], []
            for h in range(HPC):
                knat = load_pool.tile([P, NT, D], F32, tag=f"knat{h}")
                nc.sync.dma_start(knat[:], k[h].rearrange("(t p) d -> p t d", p=P))
                qnat = load_pool.tile([P, NT, D], F32, tag=f"qnat{h}")
                nc.sync.dma_start(qnat[:], q[h].rearrange("(t p) d -> p t d", p=P))
                vtmp = load_pool.tile([P, NT, D], F32, tag=f"vtmp{h}")
                nc.sync.dma_start(vtmp[:], v[h].rearrange("(t p) d -> p t d", p=P))
                knats.append(knat)
                qnats.append(qnat)
                vtmps.append(vtmp)

            heads = [{} for _ in range(HPC)]

            def setup_chunks(h):
                """Emission chunks for head h's setup, in dependency order."""
                st = heads[h]

                def scale_chain():
                    knat, vtmp = knats[h], vtmps[h]
                    ktmp = work_pool.tile([P, NT, D], F32, tag="ktmp")
                    nc.vector.tensor_mul(out=ktmp[:], in0=knat[:], in1=knat[:])
                    ksq = head_pool.tile([P, NT], F32, tag="ksq")
                    nc.vector.tensor_reduce(
                        ksq[:], ktmp[:],
                        axis=mybir.AxisListType.X, op=mybir.AluOpType.add,
                    )
                    w = head_pool.tile([P, NT], F32, tag="w")
                    nc.scalar.activation(
                        w[:], ksq[:], mybir.ActivationFunctionType.Exp, scale=-0.125
                    )
                    vaug = head_pool.tile([P, NT, D + 1], MM_DT, tag="vaug")
                    nc.vector.tensor_mul(
                        out=vaug[:, :, :D],
                        in0=vtmp[:],
                        in1=w[:, :, None].to_broadcast((P, NT, D)),
                    )
                    nc.vector.tensor_copy(out=vaug[:, :, D : D + 1], in_=w[:, :, None])
                    st["vaug"] = vaug
                    st["kt"] = head_pool.tile([D, NT, P], MM_DT, tag="kt", name="kt")
                    st["qt"] = head_pool.tile([D, NT, P], MM_DT, tag="qt", name="qt")

                def tr_group(which, g):
                    def run():
                        src = knats[h] if which == "kt" else qnats[h]
                        dst = heads[h][which]
                        tp = st_pool.tile([D, 4, P], F32, tag="stg", name="tp")
                        for j in range(4):
                            nc.tensor.transpose(
                                tp[:, j, :], src[:, 4 * g + j, :], identity[:]
                            )
                        nc.vector.tensor_copy(
                            out=dst[:, 4 * g : 4 * g + 4, :], in_=tp[:]
                        )

                    return run

                yield scale_chain
                # query block mb needs kt groups 0..mb and qt group mb
                for g in range(NT // 4):
                    yield tr_group("kt", g)
                    yield tr_group("qt", g)

            def emit_mb(h, mb):
                kt, qt, vaug = heads[h]["kt"], heads[h]["qt"], heads[h]["vaug"]
                nsub = 4 * mb          # sub-diagonal key tiles
                qt_mb = qt[:, 4 * mb : 4 * mb + 4, :]  # [64, 512]
                ot = ot_pool.tile([D + 1, QB], F32, tag="ot")
                ntiles = nsub + 4

                # --- sub-diagonal tiles: unmasked, accumulate first ---
                prev = None
                for s in range(0, nsub, G):
                    stg = st_pool.tile([P, G, QB], F32, tag="stg")
                    for j in range(G):
                        nc.tensor.matmul(
                            stg[:, j, :], kt[:, s + j, :], qt_mb,
                            start=True, stop=True, skip_group_check=True,
                        )
                    pg = p_pool.tile([P, G, QB], MM_DT, tag="pg")
                    nc.scalar.activation(
                        pg[:], stg[:],
                        mybir.ActivationFunctionType.Exp, scale=0.25,
                    )
                    if prev is not None:
                        _emit_pv(nc, ot, vaug, prev, ntiles)
                    prev = (pg, [s, s + 1])
                if prev is not None:
                    _emit_pv(nc, ot, vaug, prev, ntiles)

                # --- diagonal block: 4 masked tiles, accumulated last ---
                pgd = p_pool.tile([P, 4, QB], MM_DT, tag="pgd")
                for a in range(2):
                    stg = st_pool.tile([P, G, QB], F32, tag="stg")
                    for j in range(G):
                        nt = 4 * mb + 2 * a + j
                        nc.tensor.matmul(
                            stg[:, j, :], kt[:, nt, :], qt_mb,
                            start=True, stop=True, skip_group_check=True,
                        )
                    nc.scalar.activation(
                        pgd[:, 2 * a : 2 * a + 2, :], stg[:],
                        mybir.ActivationFunctionType.Exp, scale=0.25,
                    )
                    for j in range(G):
                        # keep pgd[n, jj, m] iff m - n - 128 jj >= 0
                        jj = 2 * a + j
                        nc.gpsimd.affine_select(
                            out=pgd[:, jj, :], in_=pgd[:, jj, :],
                            compare_op=mybir.AluOpType.is_ge, fill=0.0,
                            base=-P * jj, pattern=[[1, QB]],
                            channel_multiplier=-1,
                        )
                for j in range(4):
                    nc.tensor.matmul(
                        ot[:], vaug[:, 4 * mb + j, :], pgd[:, j, :],
                        start=(nsub == 0 and j == 0), stop=(j == 3),
                        skip_group_check=True,
                    )

                # ---------- epilogue: transpose + normalize ----------
                ot_sb = epi_pool.tile([D + 1, QB], F32, tag="ot_sb")
                nc.vector.tensor_copy(out=ot_sb[:], in_=ot[:])
                tpo = ot_pool.tile([P, 4, D + 1], F32, tag="ot", name="tpo")
                for j in range(4):
                    nc.tensor.transpose(
                        tpo[:, j, :],
                        ot_sb[:, j * P : (j + 1) * P],
                        identity[: D + 1, : D + 1],
                    )
                linv = epi_pool.tile([P, 4], F32, tag="linv")
                nc.vector.reciprocal(linv[:], tpo[:, :, D])
                o_sb = epi_pool.tile([P, 4, D], F32, tag="o_sb")
                for j in range(4):
                    nc.vector.tensor_scalar_mul(
                        o_sb[:, j, :], tpo[:, j, :D], linv[:, j : j + 1]
                    )
                nc.sync.dma_start(
                    out[h, mb * QB : (mb + 1) * QB, :].rearrange(
                        "(j p) d -> p j d", p=P
                    ),
                    o_sb[:],
                )

            # ---- software-pipelined emission ----
            pending = list(setup_chunks(0))
            for c in pending[:3]:  # scale chain, ktr0, qtr0
                c()
            pending = pending[3:]
            for h in range(HPC):
                if h + 1 < HPC:
                    pending += list(setup_chunks(h + 1))
                for mb in range(MBS):
                    emit_mb(h, mb)
                    if h + 1 == HPC and mb == MBS - 1:
                        take = len(pending)
                    else:
                        take = -(-len(pending) // (MBS - mb)) if pending else 0
                    for c in pending[:take]:
                        c()
                    pending = pending[take:]

    nc.compile()
    return nc


def _emit_pv(nc, ot, vaug, group, ntiles):
    pg, tiles = group
    for j, nt in enumerate(tiles):
        nc.tensor.matmul(
            ot[:],
            vaug[:, nt, :],
            pg[:, j, :],
            start=(nt == 0),
            stop=(nt == ntiles - 1),
            skip_group_check=True,
        )


_NC = None


def _get_nc():
    global _NC
    if _NC is None:
        _NC = build_nc()
    return _NC


def kernel(q: np.ndarray, k: np.ndarray, v: np.ndarray) -> np.ndarray:
    from concourse.bass_utils import run_bass_kernel_spmd

    nc = _get_nc()
    qf = np.ascontiguousarray(np.asarray(q, dtype=np.float32).reshape(B * H, N, D))
    kf = np.ascontiguousarray(np.asarray(k, dtype=np.float32).reshape(B * H, N, D))
    vf = np.ascontiguousarray(np.asarray(v, dtype=np.float32).reshape(B * H, N, D))
    in_maps = [
        {
            "q": np.ascontiguousarray(qf[c * HPC : (c + 1) * HPC]),
            "k": np.ascontiguousarray(kf[c * HPC : (c + 1) * HPC]),
            "v": np.ascontiguousarray(vf[c * HPC : (c + 1) * HPC]),
        }
        for c in range(NCORES)
    ]
    res = run_bass_kernel_spmd(nc, in_maps, core_ids=list(range(NCORES)))
    outs = [res.results[c]["out"] for c in range(NCORES)]
    return np.concatenate(outs, axis=0).reshape(B, H, N, D)


if __name__ == "__main__":
    rng = np.random.default_rng(0)
    qq = rng.standard_normal((B, H, N, D), dtype=np.float32)
    kk = rng.standard_normal((B, H, N, D), dtype=np.float32)
    vv = rng.standard_normal((B, H, N, D), dtype=np.float32)
    o = kernel(q=qq, k=kk, v=vv)
    print("kernel ran, out shape", o.shape, "finite:", np.isfinite(o).all())
